# revision 6
# baseline (speedup 1.0000x reference)
"""CrossCoderDecoder kernel for 8x Trainium2 NeuronCores.

Computes out[b, l, d] = sum_f f[b, f] * W[l, f, d] + bias[l, d]
(einsum 'bf,lfd->bld' + bias) for B=2048, L=2, F=65536, D=768, fp32 I/O.

Strategy:
  - Shard the contraction axis F across the 8 cores (8192 columns each).
  - Host side: slice + transpose f to fT [F_s, B], cast f/W shards to fp16
    (11-bit significand -> ~3e-4 rel err on this problem, 8x better than
    bf16 at the same DMA and TensorE cost; PSUM accumulates in fp32).
  - Device side (per core): out[l] = fT.T @ W[l] as two chained
    matmul_tile_kernel calls inside one TileContext (K=8192 contraction,
    M=2048 on PSUM partitions, N=768 free).
  - Host side: sum the 8 partial [L, B, D] outputs, add bias, reorder to
    [B, L, D].
"""

import numpy as np

B, L, F, D = 2048, 2, 65536, 768
NCORES = 8
FS = F // NCORES  # 8192 per-core contraction slice

_NC_CACHE = None

# v2 geometry
KT = FS // 128          # 64 k-tiles of 128 (contraction)
BH = B // 2             # 1024 b per half (fT half resident in SBUF)
NB = 512                # moving free dim per matmul
DG = D // 256           # 3 d-groups of 256 (2 stationary subtiles each)


def _build_v2():
    """Custom kernel: out[l, d, b] = sum_k W[l, k, d] * fT[k, b].

    Per b-half (1024 cols of fT resident in SBUF as 64 [128,1024] tiles):
    for each (l, d-group of 256): stream W k-tiles [128,256]; each 128-col
    stationary slice serves two N=512 matmuls (b subchunks), accumulating
    over all 64 k-tiles into 4 PSUM banks; evacuate via scalar/vector copy
    and DMA to out[l, d, b].
    """
    import concourse.mybir as mybir
    import concourse.tile as tile
    from concourse import bacc

    F16, F32 = mybir.dt.float16, mybir.dt.float32
    nc = bacc.Bacc("TRN2", target_bir_lowering=False, num_devices=NCORES)
    fT = nc.dram_tensor("fT", [FS, B], F16, kind="ExternalInput")
    Wt = nc.dram_tensor("Wt", [L, FS, D], F16, kind="ExternalInput")
    out = nc.dram_tensor("out", [L, D, B], F32, kind="ExternalOutput")
    fT_ap, Wt_ap, out_ap = fT.ap(), Wt.ap(), out.ap()

    with tile.TileContext(nc) as tc:
        with (
            tc.tile_pool(name="fpool", bufs=1) as fpool,
            tc.tile_pool(name="wpool", bufs=24) as wpool,
            tc.tile_pool(name="opool", bufs=8) as opool,
            tc.tile_pool(name="pp", bufs=8, space="PSUM") as pp,
        ):
            for bh in range(2):
                # fT half: 64 resident tiles; DMAs issued lazily inside the
                # first sweep's k-loop so the W stream is never stuck behind
                # them, and the previous half's tiles free progressively.
                fts = [fpool.tile([128, BH], F16, name=f"ft{k}")
                       for k in range(KT)]
                first_sweep = True
                for l in range(L):
                    for dg in range(DG):
                        accs = [pp.tile([128, NB], F32, name="acc")
                                for _ in range(4)]
                        for k in range(KT):
                            if first_sweep:
                                nc.sync.dma_start(
                                    fts[k][:],
                                    fT_ap[k * 128:(k + 1) * 128,
                                          bh * BH:(bh + 1) * BH],
                                )
                            wt = wpool.tile([128, 256], F16, name="wt")
                            nc.scalar.dma_start(
                                wt[:],
                                Wt_ap[l, k * 128:(k + 1) * 128,
                                      dg * 256:(dg + 1) * 256],
                            )
                            for ds in range(2):
                                for bs in range(2):
                                    nc.tensor.matmul(
                                        accs[ds * 2 + bs][:],
                                        wt[:, ds * 128:(ds + 1) * 128],
                                        fts[k][:, bs * NB:(bs + 1) * NB],
                                        start=(k == 0),
                                        stop=(k == KT - 1),
                                    )
                        first_sweep = False
                        for ds in range(2):
                            for bs in range(2):
                                ot = opool.tile([128, NB], F32, name="ot")
                                if bs == 0:
                                    nc.scalar.copy(ot[:], accs[ds * 2 + bs][:])
                                else:
                                    nc.vector.tensor_copy(ot[:], accs[ds * 2 + bs][:])
                                nc.gpsimd.dma_start(
                                    out_ap[l,
                                           dg * 256 + ds * 128:
                                           dg * 256 + (ds + 1) * 128,
                                           bh * BH + bs * NB:
                                           bh * BH + (bs + 1) * NB],
                                    ot[:],
                                )
    nc.finalize()
    return nc


def _build_v1():
    """v1: two chained production matmul_tile_kernel calls (out[l] = fT.T@W[l])."""
    import concourse.mybir as mybir
    import concourse.tile as tile
    from concourse import bacc
    from concourse.kernels.tile_matmul import matmul_tile_kernel

    nc = bacc.Bacc("TRN2", target_bir_lowering=False, num_devices=NCORES)
    fT = nc.dram_tensor("fT", [FS, B], mybir.dt.float16, kind="ExternalInput")
    Wt = nc.dram_tensor("Wt", [L, FS, D], mybir.dt.float16, kind="ExternalInput")
    out = nc.dram_tensor("out", [L, B, D], mybir.dt.float32, kind="ExternalOutput")

    with tile.TileContext(nc) as tc:
        for l in range(L):
            matmul_tile_kernel(
                tc,
                kxm_ap=fT.ap(),
                kxn_ap=Wt.ap()[l],
                mxn_ap=out.ap()[l],
            )
    nc.finalize()
    return nc


VERSION = 2


def _build():
    global _NC_CACHE
    if _NC_CACHE is not None:
        return _NC_CACHE
    _NC_CACHE = _build_v2() if VERSION == 2 else _build_v1()
    return _NC_CACHE


def _shard_inputs(f, W):
    in_maps = []
    for s in range(NCORES):
        sl = slice(s * FS, (s + 1) * FS)
        fTs = np.ascontiguousarray(f[:, sl].T).astype(np.float16)
        Ws = np.ascontiguousarray(W[:, sl, :]).astype(np.float16)
        in_maps.append({"fT": fTs, "Wt": Ws})
    return in_maps


def run(f, W, bias, trace=False):
    """Run on hardware; returns (full output [B, L, D] fp32, BassKernelResults)."""
    from concourse.bass_utils import run_bass_kernel_spmd

    nc = _build()
    res = run_bass_kernel_spmd(
        nc, _shard_inputs(f, W), core_ids=list(range(NCORES)), trace=trace
    )
    shape = (L, D, B) if VERSION == 2 else (L, B, D)
    acc = np.zeros(shape, np.float32)
    for r in res.results:
        acc += r["out"]
    if VERSION == 2:
        out = acc.transpose(2, 0, 1) + np.asarray(bias, np.float32)[None, :, :]
    else:
        out = acc.transpose(1, 0, 2) + np.asarray(bias, np.float32)[None, :, :]
    return np.ascontiguousarray(out), res


def kernel(f, W, bias):
    out, _ = run(np.asarray(f), np.asarray(W), np.asarray(bias))
    return out


# revision 8
# speedup vs baseline: 1.0453x; 1.0453x over previous
"""CrossCoderDecoder kernel for 8x Trainium2 NeuronCores.

Computes out[b, l, d] = sum_f f[b, f] * W[l, f, d] + bias[l, d]
(einsum 'bf,lfd->bld' + bias) for B=2048, L=2, F=65536, D=768, fp32 I/O.

Strategy:
  - Shard the contraction axis F across the 8 cores (8192 columns each).
  - Host side: slice + transpose f to fT [F_s, B], cast f/W shards to fp16
    (11-bit significand -> ~3e-4 rel err on this problem, 8x better than
    bf16 at the same DMA and TensorE cost; PSUM accumulates in fp32).
  - Device side (per core): out[l] = fT.T @ W[l] as two chained
    matmul_tile_kernel calls inside one TileContext (K=8192 contraction,
    M=2048 on PSUM partitions, N=768 free).
  - Host side: sum the 8 partial [L, B, D] outputs, add bias, reorder to
    [B, L, D].
"""

import numpy as np

B, L, F, D = 2048, 2, 65536, 768
NCORES = 8
FS = F // NCORES  # 8192 per-core contraction slice

_NC_CACHE = None

# v2 geometry
KT = FS // 128          # 64 k-tiles of 128 (contraction)
BH = B // 2             # 1024 b per half (fT half resident in SBUF)
NB = 512                # moving free dim per matmul
DG = D // 256           # 3 d-groups of 256 (2 stationary subtiles each)


def _build_v2():
    """Custom kernel: out[l, d, b] = sum_k W[l, k, d] * fT[k, b].

    Per b-half (1024 cols of fT resident in SBUF as 64 [128,1024] tiles):
    for each (l, d-group of 256): stream W k-tiles [128,256]; each 128-col
    stationary slice serves two N=512 matmuls (b subchunks), accumulating
    over all 64 k-tiles into 4 PSUM banks; evacuate via scalar/vector copy
    and DMA to out[l, d, b].
    """
    import concourse.mybir as mybir
    import concourse.tile as tile
    from concourse import bacc

    F16, F32 = mybir.dt.float16, mybir.dt.float32
    nc = bacc.Bacc("TRN2", target_bir_lowering=False, num_devices=NCORES)
    fT = nc.dram_tensor("fT", [FS, B], F16, kind="ExternalInput")
    Wt = nc.dram_tensor("Wt", [L, FS, D], F16, kind="ExternalInput")
    out = nc.dram_tensor("out", [L, D, B], F32, kind="ExternalOutput")
    fT_ap, Wt_ap, out_ap = fT.ap(), Wt.ap(), out.ap()

    with tile.TileContext(nc) as tc:
        with (
            tc.tile_pool(name="fpool", bufs=1) as fpool,
            tc.tile_pool(name="wpool", bufs=8) as wpool,
            tc.tile_pool(name="opool", bufs=8) as opool,
            tc.tile_pool(name="pp", bufs=8, space="PSUM") as pp,
        ):
            FG = 4   # k-tiles per fT DMA group (1 MiB per DMA)
            WG = 4   # k-tiles per W DMA group (256 KiB per DMA)
            for bh in range(2):
                # fT half resident as 16 groups of 4 k-tiles; DMAs issued
                # lazily inside the first sweep's k-loop so the W stream is
                # never stuck behind them, and the previous half's slots
                # free progressively.
                ftg = [fpool.tile([128, FG, BH], F16, name=f"ftg{g}")
                       for g in range(KT // FG)]
                first_sweep = True
                for l in range(L):
                    for dg in range(DG):
                        accs = [pp.tile([128, NB], F32, name="acc")
                                for _ in range(4)]
                        wt = None
                        for k in range(KT):
                            g, gi = divmod(k, FG)
                            if first_sweep and gi == 0:
                                nc.sync.dma_start(
                                    ftg[g][:],
                                    fT_ap[g * FG * 128:(g + 1) * FG * 128,
                                          bh * BH:(bh + 1) * BH]
                                    .rearrange("(c p) b -> p c b", p=128),
                                )
                            if k % WG == 0:
                                wt = wpool.tile([128, WG, 256], F16, name="wt")
                                nc.scalar.dma_start(
                                    wt[:],
                                    Wt_ap[l, k * 128:(k + WG) * 128,
                                          dg * 256:(dg + 1) * 256]
                                    .rearrange("(c p) d -> p c d", p=128),
                                )
                            for ds in range(2):
                                for bs in range(2):
                                    nc.tensor.matmul(
                                        accs[ds * 2 + bs][:],
                                        wt[:, k % WG, ds * 128:(ds + 1) * 128],
                                        ftg[g][:, gi, bs * NB:(bs + 1) * NB],
                                        start=(k == 0),
                                        stop=(k == KT - 1),
                                    )
                        first_sweep = False
                        for ds in range(2):
                            for bs in range(2):
                                ot = opool.tile([128, NB], F32, name="ot")
                                if bs == 0:
                                    nc.scalar.copy(ot[:], accs[ds * 2 + bs][:])
                                else:
                                    nc.vector.tensor_copy(ot[:], accs[ds * 2 + bs][:])
                                nc.gpsimd.dma_start(
                                    out_ap[l,
                                           dg * 256 + ds * 128:
                                           dg * 256 + (ds + 1) * 128,
                                           bh * BH + bs * NB:
                                           bh * BH + (bs + 1) * NB],
                                    ot[:],
                                )
    nc.finalize()
    return nc


def _build_v1():
    """v1: two chained production matmul_tile_kernel calls (out[l] = fT.T@W[l])."""
    import concourse.mybir as mybir
    import concourse.tile as tile
    from concourse import bacc
    from concourse.kernels.tile_matmul import matmul_tile_kernel

    nc = bacc.Bacc("TRN2", target_bir_lowering=False, num_devices=NCORES)
    fT = nc.dram_tensor("fT", [FS, B], mybir.dt.float16, kind="ExternalInput")
    Wt = nc.dram_tensor("Wt", [L, FS, D], mybir.dt.float16, kind="ExternalInput")
    out = nc.dram_tensor("out", [L, B, D], mybir.dt.float32, kind="ExternalOutput")

    with tile.TileContext(nc) as tc:
        for l in range(L):
            matmul_tile_kernel(
                tc,
                kxm_ap=fT.ap(),
                kxn_ap=Wt.ap()[l],
                mxn_ap=out.ap()[l],
            )
    nc.finalize()
    return nc


VERSION = 2


def _build():
    global _NC_CACHE
    if _NC_CACHE is not None:
        return _NC_CACHE
    _NC_CACHE = _build_v2() if VERSION == 2 else _build_v1()
    return _NC_CACHE


def _shard_inputs(f, W):
    in_maps = []
    for s in range(NCORES):
        sl = slice(s * FS, (s + 1) * FS)
        fTs = np.ascontiguousarray(f[:, sl].T).astype(np.float16)
        Ws = np.ascontiguousarray(W[:, sl, :]).astype(np.float16)
        in_maps.append({"fT": fTs, "Wt": Ws})
    return in_maps


def run(f, W, bias, trace=False):
    """Run on hardware; returns (full output [B, L, D] fp32, BassKernelResults)."""
    from concourse.bass_utils import run_bass_kernel_spmd

    nc = _build()
    res = run_bass_kernel_spmd(
        nc, _shard_inputs(f, W), core_ids=list(range(NCORES)), trace=trace
    )
    shape = (L, D, B) if VERSION == 2 else (L, B, D)
    acc = np.zeros(shape, np.float32)
    for r in res.results:
        acc += r["out"]
    if VERSION == 2:
        out = acc.transpose(2, 0, 1) + np.asarray(bias, np.float32)[None, :, :]
    else:
        out = acc.transpose(1, 0, 2) + np.asarray(bias, np.float32)[None, :, :]
    return np.ascontiguousarray(out), res


def kernel(f, W, bias):
    out, _ = run(np.asarray(f), np.asarray(W), np.asarray(bias))
    return out


# revision 10
# speedup vs baseline: 1.0482x; 1.0027x over previous
"""CrossCoderDecoder kernel for 8x Trainium2 NeuronCores.

Computes out[b, l, d] = sum_f f[b, f] * W[l, f, d] + bias[l, d]
(einsum 'bf,lfd->bld' + bias) for B=2048, L=2, F=65536, D=768, fp32 I/O.

Strategy:
  - Shard the contraction axis F across the 8 cores (8192 columns each).
  - Host side: slice + transpose f to fT [F_s, B], cast f/W shards to fp16
    (11-bit significand -> ~3e-4 rel err on this problem, 8x better than
    bf16 at the same DMA and TensorE cost; PSUM accumulates in fp32).
  - Device side (per core): out[l] = fT.T @ W[l] as two chained
    matmul_tile_kernel calls inside one TileContext (K=8192 contraction,
    M=2048 on PSUM partitions, N=768 free).
  - Host side: sum the 8 partial [L, B, D] outputs, add bias, reorder to
    [B, L, D].
"""

import numpy as np

B, L, F, D = 2048, 2, 65536, 768
NCORES = 8
FS = F // NCORES  # 8192 per-core contraction slice

_NC_CACHE = None

# v2 geometry
KT = FS // 128          # 64 k-tiles of 128 (contraction)
BH = B // 2             # 1024 b per half (fT half resident in SBUF)
NB = 512                # moving free dim per matmul
DG = D // 256           # 3 d-groups of 256 (2 stationary subtiles each)


def _build_v2():
    """Custom kernel: out[l, d, b] = sum_k W[l, k, d] * fT[k, b].

    Per b-half (1024 cols of fT resident in SBUF as 64 [128,1024] tiles):
    for each (l, d-group of 256): stream W k-tiles [128,256]; each 128-col
    stationary slice serves two N=512 matmuls (b subchunks), accumulating
    over all 64 k-tiles into 4 PSUM banks; evacuate via scalar/vector copy
    and DMA to out[l, d, b].
    """
    import concourse.mybir as mybir
    import concourse.tile as tile
    from concourse import bacc

    F16, F32 = mybir.dt.float16, mybir.dt.float32
    nc = bacc.Bacc("TRN2", target_bir_lowering=False, num_devices=NCORES)
    fT = nc.dram_tensor("fT", [FS, B], F16, kind="ExternalInput")
    Wt = nc.dram_tensor("Wt", [L, FS, D], F16, kind="ExternalInput")
    out = nc.dram_tensor("out", [L, D, B], F32, kind="ExternalOutput")
    fT_ap, Wt_ap, out_ap = fT.ap(), Wt.ap(), out.ap()

    with tile.TileContext(nc) as tc:
        with (
            tc.tile_pool(name="fpool", bufs=1) as fpool,
            tc.tile_pool(name="wpool", bufs=8) as wpool,
            tc.tile_pool(name="opool", bufs=8) as opool,
            tc.tile_pool(name="pp", bufs=8, space="PSUM") as pp,
        ):
            # fT DMA group sizes (k-tiles per DMA): first groups small so the
            # first matmuls aren't gated on a large transfer, then 1 MiB.
            FGS = [1, 3] + [4] * 15
            FG_START = [0]
            for s in FGS:
                FG_START.append(FG_START[-1] + s)
            K2G = {}
            for g, s in enumerate(FGS):
                for gi in range(s):
                    K2G[FG_START[g] + gi] = (g, gi)
            WG = 4   # k-tiles per W DMA group (256 KiB per DMA)
            for bh in range(2):
                # fT half resident; DMAs issued lazily inside the first
                # sweep's k-loop so the W stream is never stuck behind
                # them, and the previous half's slots free progressively.
                ftg = [fpool.tile([128, s, BH], F16, name=f"ftg{g}")
                       for g, s in enumerate(FGS)]
                first_sweep = True
                for l in range(L):
                    for dg in range(DG):
                        accs = [pp.tile([128, NB], F32, name="acc")
                                for _ in range(4)]
                        wt = None
                        for k in range(KT):
                            g, gi = K2G[k]
                            if first_sweep and gi == 0:
                                nc.sync.dma_start(
                                    ftg[g][:],
                                    fT_ap[FG_START[g] * 128:
                                          FG_START[g + 1] * 128,
                                          bh * BH:(bh + 1) * BH]
                                    .rearrange("(c p) b -> p c b", p=128),
                                )
                            if k % WG == 0:
                                wt = wpool.tile([128, WG, 256], F16, name="wt")
                                nc.scalar.dma_start(
                                    wt[:],
                                    Wt_ap[l, k * 128:(k + WG) * 128,
                                          dg * 256:(dg + 1) * 256]
                                    .rearrange("(c p) d -> p c d", p=128),
                                )
                            for ds in range(2):
                                for bs in range(2):
                                    nc.tensor.matmul(
                                        accs[ds * 2 + bs][:],
                                        wt[:, k % WG, ds * 128:(ds + 1) * 128],
                                        ftg[g][:, gi, bs * NB:(bs + 1) * NB],
                                        start=(k == 0),
                                        stop=(k == KT - 1),
                                    )
                        first_sweep = False
                        for ds in range(2):
                            for bs in range(2):
                                ot = opool.tile([128, NB], F32, name="ot")
                                if bs == 0:
                                    nc.scalar.copy(ot[:], accs[ds * 2 + bs][:])
                                else:
                                    nc.vector.tensor_copy(ot[:], accs[ds * 2 + bs][:])
                                nc.sync.dma_start(
                                    out_ap[l,
                                           dg * 256 + ds * 128:
                                           dg * 256 + (ds + 1) * 128,
                                           bh * BH + bs * NB:
                                           bh * BH + (bs + 1) * NB],
                                    ot[:],
                                )
    nc.finalize()
    return nc


def _build_v1():
    """v1: two chained production matmul_tile_kernel calls (out[l] = fT.T@W[l])."""
    import concourse.mybir as mybir
    import concourse.tile as tile
    from concourse import bacc
    from concourse.kernels.tile_matmul import matmul_tile_kernel

    nc = bacc.Bacc("TRN2", target_bir_lowering=False, num_devices=NCORES)
    fT = nc.dram_tensor("fT", [FS, B], mybir.dt.float16, kind="ExternalInput")
    Wt = nc.dram_tensor("Wt", [L, FS, D], mybir.dt.float16, kind="ExternalInput")
    out = nc.dram_tensor("out", [L, B, D], mybir.dt.float32, kind="ExternalOutput")

    with tile.TileContext(nc) as tc:
        for l in range(L):
            matmul_tile_kernel(
                tc,
                kxm_ap=fT.ap(),
                kxn_ap=Wt.ap()[l],
                mxn_ap=out.ap()[l],
            )
    nc.finalize()
    return nc


VERSION = 2


def _build():
    global _NC_CACHE
    if _NC_CACHE is not None:
        return _NC_CACHE
    _NC_CACHE = _build_v2() if VERSION == 2 else _build_v1()
    return _NC_CACHE


def _shard_inputs(f, W):
    in_maps = []
    for s in range(NCORES):
        sl = slice(s * FS, (s + 1) * FS)
        fTs = np.ascontiguousarray(f[:, sl].T).astype(np.float16)
        Ws = np.ascontiguousarray(W[:, sl, :]).astype(np.float16)
        in_maps.append({"fT": fTs, "Wt": Ws})
    return in_maps


def run(f, W, bias, trace=False):
    """Run on hardware; returns (full output [B, L, D] fp32, BassKernelResults)."""
    from concourse.bass_utils import run_bass_kernel_spmd

    nc = _build()
    res = run_bass_kernel_spmd(
        nc, _shard_inputs(f, W), core_ids=list(range(NCORES)), trace=trace
    )
    shape = (L, D, B) if VERSION == 2 else (L, B, D)
    acc = np.zeros(shape, np.float32)
    for r in res.results:
        acc += r["out"]
    if VERSION == 2:
        out = acc.transpose(2, 0, 1) + np.asarray(bias, np.float32)[None, :, :]
    else:
        out = acc.transpose(1, 0, 2) + np.asarray(bias, np.float32)[None, :, :]
    return np.ascontiguousarray(out), res


def kernel(f, W, bias):
    out, _ = run(np.asarray(f), np.asarray(W), np.asarray(bias))
    return out
